# revision 7
# baseline (speedup 1.0000x reference)
"""Distributed GQA attention (B=2,T=2048,C=2048,H=16,KV=4,D=128, RoPE, causal)
for one TRN2 chip (8 NeuronCores).

Sharding (single AllGather): core c -> batch b=c//4, stripe s=c%4.
Each core handles query rows {r : r % 4 == s} of its batch (512 rows,
interleaved so causal spans are shape-uniform across cores -> one SPMD graph),
computes K/V for a 512-token chunk (AllGather across the 4-core batch group),
and produces complete output rows. Host reassembles by stripe.

Per-core pipeline (PE-bound; ~960 matmuls of N=512):
  barrier AllReduce first (absorbs the cross-core rendezvous during KVproj)
  KVproj bf16 -> K^T[d,t] (+RoPE), V[t,d] -> AllGather (input on ACT DMA queue)
  Qproj bf16 (+RoPE/sqrt(D) via cast-scale on ACT)
  attention, scores transposed: S^T[k,(h4,q)], two k-tiles per PSUM pair,
    one Exp per pair (ACT), multiplicative bf16 staircase mask (DVE),
    AV accumulation on PE, softmax denominators via DVE pair-tree +
    gpsimd partition_all_reduce (no PE matmuls, no PSUM bank)
  Oproj interleaved per q-tile (wo fully resident), outputs on SP DMA queue.
"""

import numpy as np
import ml_dtypes

import concourse.bass as bass
import concourse.tile as tile
from concourse import bacc, bass_isa, mybir
from concourse.bass_utils import run_bass_kernel_spmd

B, T, C = 2, 2048, 2048
H, KV, D = 16, 4, 128
G4 = H // KV            # q heads per kv head
THETA = 10000.0
P = 128
CT = C // P             # 16 c-tiles
TQ = 512                # queries per core
NQT = TQ // P           # 4 q-tiles
NTT = T // P            # 16 token tiles
NR = 4                  # ranks per batch group

f32 = mybir.dt.float32
bf16 = mybir.dt.bfloat16

_compiled = {}


def _build():
    nc = bacc.Bacc("TRN2", target_bir_lowering=False, debug=False, num_devices=8)
    xq_e = nc.dram_tensor("xq", [P, CT * TQ], bf16, kind="ExternalInput")
    xkv_e = nc.dram_tensor("xkv", [P, CT * TQ], bf16, kind="ExternalInput")  # chunk, pre-tiled [p,(ct t)]
    wq_e = nc.dram_tensor("wq", [P, H * CT * D], bf16, kind="ExternalInput")  # [p,(h ct d)]
    wkv_e = nc.dram_tensor("wkv", [P, CT * 2 * KV * D], bf16, kind="ExternalInput")  # [p,(ct n)]
    wo_e = nc.dram_tensor("wo", [P, (C // 512) * H * 512], bf16, kind="ExternalInput")  # [p,(cc hh c)]
    cq_e = nc.dram_tensor("cos_q", [D, TQ], bf16, kind="ExternalInput")
    sq_e = nc.dram_tensor("sin_q", [D, TQ], bf16, kind="ExternalInput")
    ck_e = nc.dram_tensor("cos_k", [D, TQ], bf16, kind="ExternalInput")  # chunk positions
    sk_e = nc.dram_tensor("sin_k", [D, TQ], bf16, kind="ExternalInput")
    mk_e = nc.dram_tensor("mask", [P, NQT * G4 * P], bf16, kind="ExternalInput")  # [p,(ktl h q)] 1/0
    out_e = nc.dram_tensor("out", [TQ, C], f32, kind="ExternalOutput")

    inv = 1.0 / np.sqrt(D)

    from contextlib import ExitStack

    with tile.TileContext(nc) as tc, ExitStack() as top:
        persist = top.enter_context(tc.tile_pool(name="persist", bufs=1))

        mask_t = persist.tile([P, NQT, G4 * P], bf16)
        nc.scalar.dma_start(mask_t[:], mk_e.ap().rearrange("p (kt q) -> p kt q", kt=NQT))
        ones_col = persist.tile([P, 1], bf16)
        nc.vector.memset(ones_col[:], 1.0)
        qhat = persist.tile([D, H, TQ], bf16)
        khat = persist.tile([D, NR, KV, TQ], bf16)
        vsb = persist.tile([P, NTT, KV * D], bf16)
        yhat = persist.tile([D, H, TQ], bf16)

        # Q operand pools (DMAs emitted after the KV-chunk inputs so the KV
        # inputs get queue-1 bandwidth first).
        qstack = ExitStack()
        xqp = qstack.enter_context(tc.tile_pool(name="xqp", bufs=1))
        wstream = qstack.enter_context(tc.tile_pool(name="wqstream", bufs=4))
        tabq = qstack.enter_context(tc.tile_pool(name="tabq", bufs=1))

        # ---- KV chunk projection (bf16) + AllGather ---------------------
        with tc.tile_pool(name="kvchunk", bufs=1) as kvc, \
             tc.tile_pool(name="dram", bufs=1, space="DRAM") as dram, \
             tc.tile_pool(name="ps_kvp", bufs=3, space="PSUM") as ps_kv, \
             tc.tile_pool(name="ropek", bufs=2) as ropekp:
            # per-c-tile tiles so the first KV matmuls start as soon as the
            # first chunks land (exact per-tile deps).
            xkv_r = xkv_e.ap().rearrange("p (ct t) -> p ct t", ct=CT)
            wkv_r = wkv_e.ap().rearrange("p (ct n) -> p ct n", ct=CT)
            xkvs, wks = [], []
            for ct in range(CT):
                xt = kvc.tile([P, TQ], bf16, name=f"xkv{ct}")
                nc.sync.dma_start(xt[:], xkv_r[:, ct])
                xkvs.append(xt)
                wt = kvc.tile([P, KV * D], bf16, name=f"wk{ct}")
                nc.sync.dma_start(wt[:], wkv_r[:, ct, 0:KV * D])
                wks.append(wt)
            wkvv = kvc.tile([P, CT, KV * D], bf16)
            nc.sync.dma_start(wkvv[:], wkv_r[:, :, KV * D:2 * KV * D])
            cos_k = kvc.tile([D, TQ], bf16)
            nc.scalar.dma_start(cos_k[:], ck_e.ap())
            sin_k = kvc.tile([D, TQ], bf16)
            nc.scalar.dma_start(sin_k[:], sk_e.ap())

            kchunk = kvc.tile([D, KV, TQ], bf16)
            vchunk = kvc.tile([P, NQT, KV * D], bf16)

            for g in range(KV):
                ps = ps_kv.tile([P, TQ], f32, tag="ps_kv")
                for ct in range(CT):
                    nc.tensor.matmul(ps[:], wks[ct][:, g * D:(g + 1) * D],
                                     xkvs[ct][:],
                                     start=(ct == 0), stop=(ct == CT - 1))
                kb = ropekp.tile([D, TQ], bf16, tag="rope_kb")
                nc.scalar.copy(kb[:], ps[:])
                tmp = ropekp.tile([D, TQ], bf16, tag="rope_kt")
                nc.vector.tensor_copy(tmp[0:64, :], kb[64:128, :])
                nc.vector.tensor_copy(tmp[64:128, :], kb[0:64, :])
                ksl = kchunk[:, g, :]
                nc.vector.tensor_mul(ksl, kb[:], cos_k[:])
                nc.vector.tensor_mul(tmp[:], tmp[:], sin_k[:])
                nc.vector.tensor_add(ksl, ksl, tmp[:])

            for ttl in range(NQT):
                ps = ps_kv.tile([P, KV * D], f32, tag="ps_kv")
                for ct in range(CT):
                    nc.tensor.matmul(ps[:], xkvs[ct][:, ttl * P:(ttl + 1) * P],
                                     wkvv[:, ct, :],
                                     start=(ct == 0), stop=(ct == CT - 1))
                nc.scalar.copy(vchunk[:, ttl, :], ps[:])

            # Q-side input DMAs (behind KV inputs on queue 1)
            xq = xqp.tile([P, CT, TQ], bf16)
            nc.sync.dma_start(xq[:], xq_e.ap().rearrange("p (ct q) -> p ct q", ct=CT))
            cos_q = tabq.tile([D, TQ], bf16)
            nc.scalar.dma_start(cos_q[:], cq_e.ap())
            sin_q = tabq.tile([D, TQ], bf16)
            nc.scalar.dma_start(sin_q[:], sq_e.ap())

            cc_in = dram.tile([2, P, KV, TQ], bf16)
            cc_out = dram.tile([NR, 2, P, KV, TQ], bf16)
            nc.scalar.dma_start(cc_in[0], kchunk[:])
            nc.scalar.dma_start(cc_in[1], vchunk[:].rearrange("p t n -> p (t n)").rearrange("p (g x) -> p g x", g=KV))
            nc.gpsimd.collective_compute(
                "AllGather",
                mybir.AluOpType.bypass,
                replica_groups=[[0, 1, 2, 3], [4, 5, 6, 7]],
                ins=[cc_in[:].opt()],
                outs=[cc_out[:].opt()],
            )
            # per-rank readback so attention q-tile r can start as soon as
            # rank r's K lands (qt=r needs only k-tiles of ranks 0..r)
            for r in range(NR):
                # khat[d, r, g, t] <- cc_out[r, 0, d, g, t]: contiguous 2KB runs
                nc.scalar.dma_start(khat[:, r], cc_out[r, 0])
                # vsb[p, (r ttl), n] <- cc_out[r, 1, p, ttl, n]
                nc.scalar.dma_start(
                    vsb[:, r * NQT:(r + 1) * NQT, :],
                    cc_out[r, 1].rearrange("p g x -> p (g x)").rearrange("p (ttl n) -> p ttl n", ttl=NQT))

        # ---- Q projection (bf16) ----------------------------------------
        with tc.tile_pool(name="ps_qp", bufs=3, space="PSUM") as ps_q, \
             tc.tile_pool(name="ropeq", bufs=2) as ropep:
            for h in range(H):
                wqt = wstream.tile([P, CT, D], bf16, tag="wq")
                nc.sync.dma_start(
                    wqt[:], wq_e.ap().rearrange("p (h ct d) -> p h ct d", h=H, ct=CT)[:, h])
                ps = ps_q.tile([P, TQ], f32, tag="ps_q")
                for ct in range(CT):
                    nc.tensor.matmul(ps[:], wqt[:, ct, :], xq[:, ct, :],
                                     start=(ct == 0), stop=(ct == CT - 1))
                qb = ropep.tile([D, TQ], bf16, tag="rope_qb")
                nc.scalar.mul(qb[:], ps[:], inv)
                tmp = ropep.tile([D, TQ], bf16, tag="rope_qt")
                nc.vector.tensor_copy(tmp[0:64, :], qb[64:128, :])
                nc.vector.tensor_copy(tmp[64:128, :], qb[0:64, :])
                qsl = qhat[:, h, :]
                nc.vector.tensor_mul(qsl, qb[:], cos_q[:])
                nc.vector.tensor_mul(tmp[:], tmp[:], sin_q[:])
                nc.vector.tensor_add(qsl, qsl, tmp[:])
        qstack.close()

        # ---- attention (scores transposed) + interleaved Oproj ----------
        with tc.tile_pool(name="ps_s", bufs=2, space="PSUM") as ps_sp, \
             tc.tile_pool(name="ps_y", bufs=2, space="PSUM") as ps_yp, \
             tc.tile_pool(name="ps_o", bufs=1, space="PSUM") as ps_op, \
             tc.tile_pool(name="ps_den", bufs=1, space="PSUM") as ps_dp, \
             tc.tile_pool(name="ptile", bufs=12) as ptp, \
             tc.tile_pool(name="dtmp", bufs=4) as dtp, \
             tc.tile_pool(name="small", bufs=4) as small, \
             tc.tile_pool(name="wop", bufs=1) as wop, \
             tc.tile_pool(name="outp", bufs=3) as outp:
            wos = []
            for cc in range(C // 512):
                wot = wop.tile([P, H, 512], bf16, name=f"wo{cc}")
                nc.sync.dma_start(
                    wot[:], wo_e.ap().rearrange("p (cc hh c) -> p cc hh c", cc=C // 512, hh=H)[:, cc])
                wos.append(wot)

            def emit_oproj_cc(qt, cc):
                ps_o = ps_op.tile([P, 512], f32, tag="ps_o")
                for hh in range(H):
                    nc.tensor.matmul(ps_o[:], yhat[:, hh, qt * P:(qt + 1) * P],
                                     wos[cc][:, hh, :],
                                     start=(hh == 0), stop=(hh == H - 1))
                osb = outp.tile([P, 512], f32, tag="osb")
                nc.vector.tensor_copy(osb[:], ps_o[:])
                nc.sync.dma_start(out_e.ap()[qt * P:(qt + 1) * P, cc * 512:(cc + 1) * 512], osb[:])

            for qt in range(NQT):
                for g in range(KV):
                    nkt = 4 * (qt + 1)
                    npair = nkt // 2
                    qrhs = qhat[:, g * G4:(g + 1) * G4, qt * P:(qt + 1) * P]
                    ps_y = ps_yp.tile([P, G4, P], f32, tag="ps_y")
                    tacc = dtp.tile([P, 2, G4 * P], bf16, tag="tacc")

                    def emit_scores(pi):
                        ps_pair = ps_sp.tile([P, 2, G4, P], f32, tag="ps_s")
                        for j in range(2):
                            kt = 2 * pi + j
                            nc.tensor.matmul(
                                ps_pair[:, j],
                                khat[:, kt // 4, g, (kt % 4) * P:(kt % 4 + 1) * P],
                                qrhs, start=True, stop=True)
                        pt = ptp.tile([P, 2, G4, P], bf16, tag="pt", name=f"pt{pi}")
                        nc.scalar.activation(pt[:], ps_pair[:], mybir.ActivationFunctionType.Exp)
                        if 2 * pi >= nkt - 4:
                            # staircase mask, multiplicative, on gpsimd (Pool)
                            ktl = 2 * pi - (nkt - 4)
                            nc.gpsimd.tensor_mul(
                                pt[:].rearrange("p a g q -> p a (g q)"),
                                pt[:].rearrange("p a g q -> p a (g q)"),
                                mask_t[:, ktl:ktl + 2, :])
                        # denominator partial sums trail one pair behind (DVE)
                        if pi == 1:
                            nc.vector.tensor_add(
                                tacc[:], prev_pt[0][:].rearrange("p a g q -> p a (g q)"),
                                pt[:].rearrange("p a g q -> p a (g q)"))
                        elif pi > 1:
                            nc.vector.tensor_add(
                                tacc[:], tacc[:], pt[:].rearrange("p a g q -> p a (g q)"))
                        prev_pt[0] = pt
                        return pt

                    def emit_av(pi, pt):
                        for j in range(2):
                            kt = 2 * pi + j
                            nc.tensor.matmul(ps_y[:], vsb[:, kt, g * D:(g + 1) * D],
                                             pt[:, j],
                                             start=(kt == 0), stop=(kt == nkt - 1))

                    # AV lags scores by 2 pairs so the PE never waits on Exp
                    prev_pt = [None]
                    pend = []
                    for pi in range(npair):
                        pend.append((pi, emit_scores(pi)))
                        if len(pend) > 2:
                            emit_av(*pend.pop(0))
                    for item in pend:
                        emit_av(*item)

                    # fold the pair-sum halves, partition-reduce on PE
                    s_t = dtp.tile([P, G4 * P], bf16, tag="s_t")
                    nc.vector.tensor_add(s_t[:], tacc[:, 0], tacc[:, 1])
                    ps_den = ps_dp.tile([1, G4 * P], f32, tag="ps_den")
                    nc.tensor.matmul(ps_den[:], ones_col[:], s_t[:],
                                     start=True, stop=True)
                    rec = small.tile([1, G4 * P], f32, tag="rec")
                    nc.vector.reciprocal_approx_fast(rec[:], ps_den[:])
                    bc = small.tile([P, G4, P], f32, tag="bc")
                    nc.gpsimd.partition_broadcast(bc[:], rec[:])
                    ysl = yhat[:, g * G4:(g + 1) * G4, qt * P:(qt + 1) * P]
                    nc.vector.tensor_mul(ysl, ps_y[:], bc[:])

                    # Oproj lags attention by one q-tile, one 512-wide output
                    # block per group (spreads PE/DVE load, ps_o stays 1-deep)
                    if qt > 0:
                        emit_oproj_cc(qt - 1, g)
            for cc in range(C // 512):
                emit_oproj_cc(NQT - 1, cc)

    nc.compile()
    return nc


def _rope_tables():
    freqs = 1.0 / (THETA ** (np.arange(0, D, 2, dtype=np.float64) / D))
    ang = np.arange(T, dtype=np.float64)[:, None] * freqs[None, :]
    emb = np.concatenate([ang, ang], axis=-1)          # [T, D]
    return np.cos(emb), np.sin(emb)                    # [T, D] each


def _prep_inputs(x, Wq, Wkv, Wo):
    cos, sin = _rope_tables()
    sgn = np.where(np.arange(D) < D // 2, -1.0, 1.0)   # sign for shifted term
    cosT = np.ascontiguousarray(cos.T)                 # [D, T]
    sinTs = np.ascontiguousarray(sin.T) * sgn[:, None]

    # pre-tiled layouts: every DMA reads contiguous per-partition runs
    # wq [p, (h ct d)]: wq[p, h, ct, d] = Wq.T[ct*128+p, h*128+d]
    wq_t = np.ascontiguousarray(
        Wq.T.reshape(16, 128, 16, 128).transpose(1, 2, 0, 3).reshape(128, -1)
    ).astype(ml_dtypes.bfloat16)
    # wkv [p, (ct n)]: wkv[p, ct, n] = Wkv.T[ct*128+p, n]
    wkv_t = np.ascontiguousarray(
        Wkv.T.reshape(16, 128, 1024).transpose(1, 0, 2).reshape(128, -1)
    ).astype(ml_dtypes.bfloat16)
    # wo [p, (cc hh c)]: wo[p, cc, hh, c] = Wo.T[hh*128+p, cc*512+c]
    wo_t = np.ascontiguousarray(
        Wo.T.reshape(16, 128, 4, 512).transpose(1, 2, 0, 3).reshape(128, -1)
    ).astype(ml_dtypes.bfloat16)

    in_maps = []
    for c in range(8):
        b, s = c // 4, c % 4
        rows = np.arange(s, T, 4)
        xq = np.ascontiguousarray(
            x[b][rows, :].T.reshape(16, 128, 512).transpose(1, 0, 2).reshape(128, -1)
        ).astype(ml_dtypes.bfloat16)  # [p, (ct q)]
        ch = np.arange(512 * s, 512 * (s + 1))
        xkv = np.ascontiguousarray(
            x[b][ch, :].T.reshape(16, 128, 512).transpose(1, 0, 2).reshape(128, -1)
        ).astype(ml_dtypes.bfloat16)  # [p, (ct t)] chunk
        cq = np.ascontiguousarray(cosT[:, rows]).astype(ml_dtypes.bfloat16)
        sq = np.ascontiguousarray(sinTs[:, rows]).astype(ml_dtypes.bfloat16)
        # multiplicative staircase mask, transposed and head-replicated:
        # mask[p, ktl, h, q] = 1 iff key (128*ktl + p) of the 512-wide
        # diagonal window is visible to query i=q (orig row 4q+s)
        j = np.arange(TQ)[:, None]
        i = np.arange(P)[None, :]
        mask = (j <= 4 * i + s).astype(np.float32)          # [512, 128]
        mask = mask.reshape(4, 128, 128).transpose(1, 0, 2)  # [p, ktl, q]
        mask = np.broadcast_to(mask[:, :, None, :], (128, 4, G4, 128))
        mask = np.ascontiguousarray(mask.reshape(128, -1)).astype(ml_dtypes.bfloat16)
        in_maps.append({
            "xq": xq, "xkv": xkv,
            "wq": wq_t, "wkv": wkv_t, "wo": wo_t,
            "cos_q": cq, "sin_q": sq,
            "cos_k": np.ascontiguousarray(cosT[:, ch]).astype(ml_dtypes.bfloat16),
            "sin_k": np.ascontiguousarray(sinTs[:, ch]).astype(ml_dtypes.bfloat16),
            "mask": mask,
        })
    return in_maps


def _unshard(results):
    full = np.empty((B, T, C), dtype=np.float32)
    for c in range(8):
        b, s = c // 4, c % 4
        full[b, s::4, :] = results[c]["out"]
    return full


def run(x, Wq, Wkv, Wo, trace=False, trace_kwargs=None):
    import time
    if "nc" not in _compiled:
        _compiled["nc"] = _build()
    nc = _compiled["nc"]
    in_maps = _prep_inputs(np.asarray(x), np.asarray(Wq), np.asarray(Wkv), np.asarray(Wo))
    last_err = None
    for attempt in range(3):
        try:
            res = run_bass_kernel_spmd(nc, in_maps, core_ids=list(range(8)), trace=trace,
                                       **(trace_kwargs or {}))
            return _unshard(res.results), res
        except Exception as e:  # transient NRT device errors recover on retry
            last_err = e
            time.sleep(5)
    raise last_err


def kernel(x, Wq, Wkv, Wo):
    out, _ = run(x, Wq, Wkv, Wo, trace=False)
    return out


# revision 8
# speedup vs baseline: 1.4972x; 1.4972x over previous
"""Distributed GQA attention (B=2,T=2048,C=2048,H=16,KV=4,D=128, RoPE, causal)
for one TRN2 chip (8 NeuronCores).

Sharding (single AllGather): core c -> batch b=c//4, stripe s=c%4.
Each core handles query rows {r : r % 4 == s} of its batch (512 rows,
interleaved so causal spans are shape-uniform across cores -> one SPMD graph),
computes K/V for a 512-token chunk (AllGather across the 4-core batch group),
and produces complete output rows. Host reassembles by stripe.

Per-core pipeline (PE-bound; ~960 matmuls of N=512):
  barrier AllReduce first (absorbs the cross-core rendezvous during KVproj)
  KVproj bf16 -> K^T[d,t] (+RoPE), V[t,d] -> AllGather (input on ACT DMA queue)
  Qproj bf16 (+RoPE/sqrt(D) via cast-scale on ACT)
  attention, scores transposed: S^T[k,(h4,q)], two k-tiles per PSUM pair,
    one Exp per pair (ACT), multiplicative bf16 staircase mask (DVE),
    AV accumulation on PE, softmax denominators via DVE pair-tree +
    gpsimd partition_all_reduce (no PE matmuls, no PSUM bank)
  Oproj interleaved per q-tile (wo fully resident), outputs on SP DMA queue.
"""

import numpy as np
import ml_dtypes

import concourse.bass as bass
import concourse.tile as tile
from concourse import bacc, bass_isa, mybir
from concourse.bass_utils import run_bass_kernel_spmd

B, T, C = 2, 2048, 2048
H, KV, D = 16, 4, 128
G4 = H // KV            # q heads per kv head
THETA = 10000.0
P = 128
CT = C // P             # 16 c-tiles
TQ = 512                # queries per core
NQT = TQ // P           # 4 q-tiles
NTT = T // P            # 16 token tiles
NR = 4                  # ranks per batch group

f32 = mybir.dt.float32
bf16 = mybir.dt.bfloat16

_compiled = {}


def _build():
    nc = bacc.Bacc("TRN2", target_bir_lowering=False, debug=False, num_devices=8)
    xq_e = nc.dram_tensor("xq", [P, CT * TQ], bf16, kind="ExternalInput")
    xkv_e = nc.dram_tensor("xkv", [P, CT * TQ], bf16, kind="ExternalInput")  # chunk, pre-tiled [p,(ct t)]
    wq_e = nc.dram_tensor("wq", [P, H * CT * D], bf16, kind="ExternalInput")  # [p,(h ct d)]
    wkv_e = nc.dram_tensor("wkv", [P, CT * 2 * KV * D], bf16, kind="ExternalInput")  # [p,(ct n)]
    wo_e = nc.dram_tensor("wo", [P, (C // 512) * H * 512], bf16, kind="ExternalInput")  # [p,(cc hh c)]
    cq_e = nc.dram_tensor("cos_q", [D, TQ], bf16, kind="ExternalInput")
    sq_e = nc.dram_tensor("sin_q", [D, TQ], bf16, kind="ExternalInput")
    ck_e = nc.dram_tensor("cos_k", [D, TQ], bf16, kind="ExternalInput")  # chunk positions
    sk_e = nc.dram_tensor("sin_k", [D, TQ], bf16, kind="ExternalInput")
    mk_e = nc.dram_tensor("mask", [P, NQT * G4 * P], bf16, kind="ExternalInput")  # [p,(ktl h q)] 1/0
    out_e = nc.dram_tensor("out", [TQ, C], f32, kind="ExternalOutput")

    inv = 1.0 / np.sqrt(D)

    from contextlib import ExitStack

    with tile.TileContext(nc) as tc, ExitStack() as top:
        persist = top.enter_context(tc.tile_pool(name="persist", bufs=1))

        mask_t = persist.tile([P, NQT, G4 * P], bf16)
        nc.scalar.dma_start(mask_t[:], mk_e.ap().rearrange("p (kt q) -> p kt q", kt=NQT))
        ones_col = persist.tile([P, 1], bf16)
        nc.vector.memset(ones_col[:], 1.0)
        qhat = persist.tile([D, H, TQ], bf16)
        khat = persist.tile([D, NR, KV, TQ], bf16)
        vsb = persist.tile([P, NTT, KV * D], bf16)
        yhat = persist.tile([D, H, TQ], bf16)

        # Q operand pools (DMAs emitted after the KV-chunk inputs so the KV
        # inputs get queue-1 bandwidth first).
        qstack = ExitStack()
        xqp = qstack.enter_context(tc.tile_pool(name="xqp", bufs=1))
        wstream = qstack.enter_context(tc.tile_pool(name="wqstream", bufs=4))
        tabq = qstack.enter_context(tc.tile_pool(name="tabq", bufs=1))

        # ---- KV chunk projection (bf16) + AllGather ---------------------
        with tc.tile_pool(name="kvchunk", bufs=1) as kvc, \
             tc.tile_pool(name="dram", bufs=1, space="DRAM") as dram, \
             tc.tile_pool(name="ps_kvp", bufs=3, space="PSUM") as ps_kv, \
             tc.tile_pool(name="ropek", bufs=2) as ropekp:
            # per-c-tile tiles so the first KV matmuls start as soon as the
            # first chunks land (exact per-tile deps).
            xkv_r = xkv_e.ap().rearrange("p (ct t) -> p ct t", ct=CT)
            wkv_r = wkv_e.ap().rearrange("p (ct n) -> p ct n", ct=CT)
            xkvs, wks = [], []
            for ct in range(CT):
                xt = kvc.tile([P, TQ], bf16, name=f"xkv{ct}")
                nc.sync.dma_start(xt[:], xkv_r[:, ct])
                xkvs.append(xt)
                wt = kvc.tile([P, KV * D], bf16, name=f"wk{ct}")
                nc.sync.dma_start(wt[:], wkv_r[:, ct, 0:KV * D])
                wks.append(wt)
            wkvv = kvc.tile([P, CT, KV * D], bf16)
            nc.sync.dma_start(wkvv[:], wkv_r[:, :, KV * D:2 * KV * D])
            cos_k = kvc.tile([D, TQ], bf16)
            nc.scalar.dma_start(cos_k[:], ck_e.ap())
            sin_k = kvc.tile([D, TQ], bf16)
            nc.scalar.dma_start(sin_k[:], sk_e.ap())

            kchunk = kvc.tile([D, KV, TQ], bf16)
            vchunk = kvc.tile([P, NQT, KV * D], bf16)

            for g in range(KV):
                ps = ps_kv.tile([P, TQ], f32, tag="ps_kv")
                for ct in range(CT):
                    nc.tensor.matmul(ps[:], wks[ct][:, g * D:(g + 1) * D],
                                     xkvs[ct][:],
                                     start=(ct == 0), stop=(ct == CT - 1))
                kb = ropekp.tile([D, TQ], bf16, tag="rope_kb")
                nc.scalar.copy(kb[:], ps[:])
                tmp = ropekp.tile([D, TQ], bf16, tag="rope_kt")
                nc.vector.tensor_copy(tmp[0:64, :], kb[64:128, :])
                nc.vector.tensor_copy(tmp[64:128, :], kb[0:64, :])
                ksl = kchunk[:, g, :]
                nc.vector.tensor_mul(ksl, kb[:], cos_k[:])
                nc.vector.tensor_mul(tmp[:], tmp[:], sin_k[:])
                nc.vector.tensor_add(ksl, ksl, tmp[:])

            for ttl in range(NQT):
                ps = ps_kv.tile([P, KV * D], f32, tag="ps_kv")
                for ct in range(CT):
                    nc.tensor.matmul(ps[:], xkvs[ct][:, ttl * P:(ttl + 1) * P],
                                     wkvv[:, ct, :],
                                     start=(ct == 0), stop=(ct == CT - 1))
                nc.scalar.copy(vchunk[:, ttl, :], ps[:])

            # Q-side input DMAs (behind KV inputs on queue 1)
            xq = xqp.tile([P, CT, TQ], bf16)
            nc.sync.dma_start(xq[:], xq_e.ap().rearrange("p (ct q) -> p ct q", ct=CT))
            cos_q = tabq.tile([D, TQ], bf16)
            nc.scalar.dma_start(cos_q[:], cq_e.ap())
            sin_q = tabq.tile([D, TQ], bf16)
            nc.scalar.dma_start(sin_q[:], sq_e.ap())

            cc_in = dram.tile([2, P, KV, TQ], bf16)
            cc_out = dram.tile([NR, 2, P, KV, TQ], bf16)
            nc.scalar.dma_start(cc_in[0], kchunk[:])
            nc.scalar.dma_start(cc_in[1], vchunk[:].rearrange("p t n -> p (t n)").rearrange("p (g x) -> p g x", g=KV))
            nc.gpsimd.collective_compute(
                "AllGather",
                mybir.AluOpType.bypass,
                replica_groups=[[0, 1, 2, 3], [4, 5, 6, 7]],
                ins=[cc_in[:].opt()],
                outs=[cc_out[:].opt()],
            )
            # per-rank readback so attention q-tile r can start as soon as
            # rank r's K lands (qt=r needs only k-tiles of ranks 0..r)
            for r in range(NR):
                # khat[d, r, g, t] <- cc_out[r, 0, d, g, t]: contiguous 2KB runs
                nc.scalar.dma_start(khat[:, r], cc_out[r, 0])
                # vsb[p, (r ttl), n] <- cc_out[r, 1, p, ttl, n]
                nc.scalar.dma_start(
                    vsb[:, r * NQT:(r + 1) * NQT, :],
                    cc_out[r, 1].rearrange("p g x -> p (g x)").rearrange("p (ttl n) -> p ttl n", ttl=NQT))

        # ---- Q projection (bf16) ----------------------------------------
        with tc.tile_pool(name="ps_qp", bufs=3, space="PSUM") as ps_q, \
             tc.tile_pool(name="ropeq", bufs=2) as ropep:
            for h in range(H):
                wqt = wstream.tile([P, CT, D], bf16, tag="wq")
                nc.sync.dma_start(
                    wqt[:], wq_e.ap().rearrange("p (h ct d) -> p h ct d", h=H, ct=CT)[:, h])
                ps = ps_q.tile([P, TQ], f32, tag="ps_q")
                for ct in range(CT):
                    nc.tensor.matmul(ps[:], wqt[:, ct, :], xq[:, ct, :],
                                     start=(ct == 0), stop=(ct == CT - 1))
                qb = ropep.tile([D, TQ], bf16, tag="rope_qb")
                nc.scalar.mul(qb[:], ps[:], inv)
                tmp = ropep.tile([D, TQ], bf16, tag="rope_qt")
                nc.vector.tensor_copy(tmp[0:64, :], qb[64:128, :])
                nc.vector.tensor_copy(tmp[64:128, :], qb[0:64, :])
                qsl = qhat[:, h, :]
                nc.vector.tensor_mul(qsl, qb[:], cos_q[:])
                nc.vector.tensor_mul(tmp[:], tmp[:], sin_q[:])
                nc.vector.tensor_add(qsl, qsl, tmp[:])
        qstack.close()

        # ---- attention (scores transposed) + interleaved Oproj ----------
        with tc.tile_pool(name="ps_s", bufs=2, space="PSUM") as ps_sp, \
             tc.tile_pool(name="ps_y", bufs=2, space="PSUM") as ps_yp, \
             tc.tile_pool(name="ps_o", bufs=1, space="PSUM") as ps_op, \
             tc.tile_pool(name="ps_den", bufs=1, space="PSUM") as ps_dp, \
             tc.tile_pool(name="ptile", bufs=12) as ptp, \
             tc.tile_pool(name="dtmp", bufs=4) as dtp, \
             tc.tile_pool(name="small", bufs=4) as small, \
             tc.tile_pool(name="wop", bufs=1) as wop, \
             tc.tile_pool(name="outp", bufs=3) as outp:
            wos = []
            for cc in range(C // 512):
                wot = wop.tile([P, H, 512], bf16, name=f"wo{cc}")
                nc.sync.dma_start(
                    wot[:], wo_e.ap().rearrange("p (cc hh c) -> p cc hh c", cc=C // 512, hh=H)[:, cc])
                wos.append(wot)

            def emit_oproj_cc(qt, cc):
                ps_o = ps_op.tile([P, 512], f32, tag="ps_o")
                for hh in range(H):
                    nc.tensor.matmul(ps_o[:], yhat[:, hh, qt * P:(qt + 1) * P],
                                     wos[cc][:, hh, :],
                                     start=(hh == 0), stop=(hh == H - 1))
                osb = outp.tile([P, 512], f32, tag="osb")
                nc.vector.tensor_copy(osb[:], ps_o[:])
                nc.sync.dma_start(out_e.ap()[qt * P:(qt + 1) * P, cc * 512:(cc + 1) * 512], osb[:])

            for qt in range(NQT):
                for g in range(KV):
                    nkt = 4 * (qt + 1)
                    npair = nkt // 2
                    qrhs = qhat[:, g * G4:(g + 1) * G4, qt * P:(qt + 1) * P]
                    ps_y = ps_yp.tile([P, G4, P], f32, tag="ps_y")
                    tacc = dtp.tile([P, 2, G4 * P], bf16, tag="tacc")

                    def emit_scores(pi):
                        ps_pair = ps_sp.tile([P, 2, G4, P], f32, tag="ps_s")
                        for j in range(2):
                            kt = 2 * pi + j
                            nc.tensor.matmul(
                                ps_pair[:, j],
                                khat[:, kt // 4, g, (kt % 4) * P:(kt % 4 + 1) * P],
                                qrhs, start=True, stop=True)
                        pt = ptp.tile([P, 2, G4, P], bf16, tag="pt", name=f"pt{pi}")
                        nc.scalar.activation(pt[:], ps_pair[:], mybir.ActivationFunctionType.Exp)
                        if 2 * pi >= nkt - 4:
                            # staircase mask, multiplicative bf16
                            ktl = 2 * pi - (nkt - 4)
                            nc.vector.tensor_mul(
                                pt[:].rearrange("p a g q -> p a (g q)"),
                                pt[:].rearrange("p a g q -> p a (g q)"),
                                mask_t[:, ktl:ktl + 2, :])
                        # denominator partial sums trail one pair behind (DVE)
                        if pi == 1:
                            nc.vector.tensor_add(
                                tacc[:], prev_pt[0][:].rearrange("p a g q -> p a (g q)"),
                                pt[:].rearrange("p a g q -> p a (g q)"))
                        elif pi > 1:
                            nc.vector.tensor_add(
                                tacc[:], tacc[:], pt[:].rearrange("p a g q -> p a (g q)"))
                        prev_pt[0] = pt
                        return pt

                    def emit_av(pi, pt):
                        for j in range(2):
                            kt = 2 * pi + j
                            nc.tensor.matmul(ps_y[:], vsb[:, kt, g * D:(g + 1) * D],
                                             pt[:, j],
                                             start=(kt == 0), stop=(kt == nkt - 1))

                    # AV lags scores by 2 pairs so the PE never waits on Exp
                    prev_pt = [None]
                    pend = []
                    for pi in range(npair):
                        pend.append((pi, emit_scores(pi)))
                        if len(pend) > 2:
                            emit_av(*pend.pop(0))
                    for item in pend:
                        emit_av(*item)

                    # fold the pair-sum halves, partition-reduce on PE
                    s_t = dtp.tile([P, G4 * P], bf16, tag="s_t")
                    nc.vector.tensor_add(s_t[:], tacc[:, 0], tacc[:, 1])
                    ps_den = ps_dp.tile([1, G4 * P], f32, tag="ps_den")
                    nc.tensor.matmul(ps_den[:], ones_col[:], s_t[:],
                                     start=True, stop=True)
                    rec = small.tile([1, G4 * P], f32, tag="rec")
                    nc.vector.reciprocal_approx_fast(rec[:], ps_den[:])
                    bc = small.tile([P, G4, P], f32, tag="bc")
                    nc.gpsimd.partition_broadcast(bc[:], rec[:])
                    ysl = yhat[:, g * G4:(g + 1) * G4, qt * P:(qt + 1) * P]
                    nc.vector.tensor_mul(ysl, ps_y[:], bc[:])

                    # Oproj lags attention by one q-tile, one 512-wide output
                    # block per group (spreads PE/DVE load, ps_o stays 1-deep)
                    if qt > 0:
                        emit_oproj_cc(qt - 1, g)
            for cc in range(C // 512):
                emit_oproj_cc(NQT - 1, cc)

    nc.compile()
    return nc


def _rope_tables():
    freqs = 1.0 / (THETA ** (np.arange(0, D, 2, dtype=np.float64) / D))
    ang = np.arange(T, dtype=np.float64)[:, None] * freqs[None, :]
    emb = np.concatenate([ang, ang], axis=-1)          # [T, D]
    return np.cos(emb), np.sin(emb)                    # [T, D] each


def _prep_inputs(x, Wq, Wkv, Wo):
    cos, sin = _rope_tables()
    sgn = np.where(np.arange(D) < D // 2, -1.0, 1.0)   # sign for shifted term
    cosT = np.ascontiguousarray(cos.T)                 # [D, T]
    sinTs = np.ascontiguousarray(sin.T) * sgn[:, None]

    # pre-tiled layouts: every DMA reads contiguous per-partition runs
    # wq [p, (h ct d)]: wq[p, h, ct, d] = Wq.T[ct*128+p, h*128+d]
    wq_t = np.ascontiguousarray(
        Wq.T.reshape(16, 128, 16, 128).transpose(1, 2, 0, 3).reshape(128, -1)
    ).astype(ml_dtypes.bfloat16)
    # wkv [p, (ct n)]: wkv[p, ct, n] = Wkv.T[ct*128+p, n]
    wkv_t = np.ascontiguousarray(
        Wkv.T.reshape(16, 128, 1024).transpose(1, 0, 2).reshape(128, -1)
    ).astype(ml_dtypes.bfloat16)
    # wo [p, (cc hh c)]: wo[p, cc, hh, c] = Wo.T[hh*128+p, cc*512+c]
    wo_t = np.ascontiguousarray(
        Wo.T.reshape(16, 128, 4, 512).transpose(1, 2, 0, 3).reshape(128, -1)
    ).astype(ml_dtypes.bfloat16)

    in_maps = []
    for c in range(8):
        b, s = c // 4, c % 4
        rows = np.arange(s, T, 4)
        xq = np.ascontiguousarray(
            x[b][rows, :].T.reshape(16, 128, 512).transpose(1, 0, 2).reshape(128, -1)
        ).astype(ml_dtypes.bfloat16)  # [p, (ct q)]
        ch = np.arange(512 * s, 512 * (s + 1))
        xkv = np.ascontiguousarray(
            x[b][ch, :].T.reshape(16, 128, 512).transpose(1, 0, 2).reshape(128, -1)
        ).astype(ml_dtypes.bfloat16)  # [p, (ct t)] chunk
        cq = np.ascontiguousarray(cosT[:, rows]).astype(ml_dtypes.bfloat16)
        sq = np.ascontiguousarray(sinTs[:, rows]).astype(ml_dtypes.bfloat16)
        # multiplicative staircase mask, transposed and head-replicated:
        # mask[p, ktl, h, q] = 1 iff key (128*ktl + p) of the 512-wide
        # diagonal window is visible to query i=q (orig row 4q+s)
        j = np.arange(TQ)[:, None]
        i = np.arange(P)[None, :]
        mask = (j <= 4 * i + s).astype(np.float32)          # [512, 128]
        mask = mask.reshape(4, 128, 128).transpose(1, 0, 2)  # [p, ktl, q]
        mask = np.broadcast_to(mask[:, :, None, :], (128, 4, G4, 128))
        mask = np.ascontiguousarray(mask.reshape(128, -1)).astype(ml_dtypes.bfloat16)
        in_maps.append({
            "xq": xq, "xkv": xkv,
            "wq": wq_t, "wkv": wkv_t, "wo": wo_t,
            "cos_q": cq, "sin_q": sq,
            "cos_k": np.ascontiguousarray(cosT[:, ch]).astype(ml_dtypes.bfloat16),
            "sin_k": np.ascontiguousarray(sinTs[:, ch]).astype(ml_dtypes.bfloat16),
            "mask": mask,
        })
    return in_maps


def _unshard(results):
    full = np.empty((B, T, C), dtype=np.float32)
    for c in range(8):
        b, s = c // 4, c % 4
        full[b, s::4, :] = results[c]["out"]
    return full


def run(x, Wq, Wkv, Wo, trace=False, trace_kwargs=None):
    import time
    if "nc" not in _compiled:
        _compiled["nc"] = _build()
    nc = _compiled["nc"]
    in_maps = _prep_inputs(np.asarray(x), np.asarray(Wq), np.asarray(Wkv), np.asarray(Wo))
    last_err = None
    for attempt in range(3):
        try:
            res = run_bass_kernel_spmd(nc, in_maps, core_ids=list(range(8)), trace=trace,
                                       **(trace_kwargs or {}))
            return _unshard(res.results), res
        except Exception as e:  # transient NRT device errors recover on retry
            last_err = e
            time.sleep(5)
    raise last_err


def kernel(x, Wq, Wkv, Wo):
    out, _ = run(x, Wq, Wkv, Wo, trace=False)
    return out
